# revision 23
# baseline (speedup 1.0000x reference)
# Trainium2 Bass kernel for NonLocalBlock (B=4, C=64, CI=32, H=W=80).
#
# Math (per batch, N = H*W = 6400):
#   u = Wu@x+bu, v = Wv@x+bv, g = Wg@x+bg           [CI, N]
#   f[n,m] = sum_c u[c,n] v[c,m]; softmax over n (axis=1 of f)
#   y[c,n] = sum_m f_sm[n,m] g[c,m];  out = Ww@y + bw + x
#
# S = v^T u (S[m,n] = f[n,m]); softmax is row-local over S's free axis.
# y = g @ softmax_rows(S).  8 cores = 4 batches x 2 halves of m.
#
# v2: the exp work (20.5M elems/core) is split between ACT and DVE.
#  - ACT-owned chunks: exact exp via activation(Exp), rowsums from accum_out.
#  - DVE-owned chunks: Schraudolph bit-trick exp at 1 elem/cycle:
#      int16(S * 128/ln2 + MAGIC)  bitcast->  bf16 ~= exp(S)
#    (bf16's exponent field absorbs the integer part; the mantissa linearly
#    interpolates 2^frac; MAGIC centers the +-3.3% sawtooth error).
#    Rowsums for these chunks come from a second bf16->bf16 bypass
#    tensor_scalar with accum_out, which runs in DVE 4x mode (0.25 cyc/elem).
#  - Softmax normalization divides by the sum of the SAME approx values, so
#    the sawtooth error partially cancels; end-to-end rel err ~3e-3.
# Per-block engine budget ~4.9-5.2us vs PE ~5.3us (S+y matmuls at 1 col/cyc
# fp16/bf16): roughly balanced, PE-bound at full clock.

import numpy as np

import concourse.bass as bass
import concourse.mybir as mybir
from concourse import bacc, tile
from concourse.bass_utils import run_bass_kernel_spmd

F32 = mybir.dt.float32
BF16 = mybir.dt.bfloat16
F16 = mybir.dt.float16
I16 = mybir.dt.int16

B, C, CI, H, W = 4, 64, 32, 80, 80
N = H * W              # 6400
NCORES = 8
MH = N // 2            # 3200 rows of S per core
MB = 128               # S row-block
NBLK = MH // MB        # 25 blocks per core
SCH = 1024             # S free-dim chunk held in PSUM (2 banks)
YCH = 512              # y free-dim chunk (1 bank)

EXP = mybir.ActivationFunctionType.Exp
A_EXP = float(np.float32(128.0 / np.log(2.0)))
B_EXP = float(np.float32(16250.875))

# chunk layout: 6x1024 + 256 tail.  Ownership tuned to measured HW costs:
# ACT-owned 1024-chunk = exp ~1121ns + accum-read ~285ns; DVE-owned =
# schraudolph ts ~1222ns (PSUM f32 in, 1x) + bypass-accum ~1210ns (the
# accumulating tensor_scalar never engages 2x/4x DVE perf modes on HW),
# so ACT rowsums are ~4x cheaper and ACT owns more columns.  ACT takes
# {0,1,4,5,tail} and DVE {2,3} so each psum ping-pong tile carries a
# balanced mix of ACT and DVE exps (the per-tile fill->exp->refill chain
# is the critical path).
# widths tuned for the per-tile chains: chunk 2 (DVE, on the 4-chunk
# tile) shrinks to 768 and the tail grows to 512, trimming the longer
# chain at slight ACT-busy cost
_W = [1024, 1024, 768, 1024, 1024, 1024, 512]
CHUNKS = []
_off = 0
for _w in _W:
    CHUNKS.append((_off, _w))
    _off += _w
assert _off == N
ACT_CHUNKS = [0, 1, 4, 5, 6]
DVE_CHUNKS = [2, 3]
NSUM = len(CHUNKS)


def _ceil_chunks(total, step):
    out = []
    off = 0
    while off < total:
        out.append((off, min(step, total - off)))
        off += step
    return out


Y_CHUNKS = _ceil_chunks(N, YCH)      # 12 x 512 + 256
U_CHUNKS = _ceil_chunks(N, 512)
V_CHUNKS = _ceil_chunks(MH, 512)


def build_nc():
    nc = bacc.Bacc("TRN2", target_bir_lowering=False, debug=False,
                   num_devices=NCORES)

    x_aug_d = nc.dram_tensor("x_aug", [C + 1, N], F16, kind="ExternalInput")
    x_m_d = nc.dram_tensor("x_m", [C + 1, MH], F16, kind="ExternalInput")
    wuT_d = nc.dram_tensor("wuT", [C + 1, CI], F16, kind="ExternalInput")
    wvT_d = nc.dram_tensor("wvT", [C + 1, CI], F16, kind="ExternalInput")
    wgT_d = nc.dram_tensor("wgT", [C + 1, CI], F16, kind="ExternalInput")
    wwT4_d = nc.dram_tensor("wwT4", [128, C], F16, kind="ExternalInput")
    resid_d = nc.dram_tensor("resid", [C, N], F32, kind="ExternalInput")
    out_d = nc.dram_tensor("out", [C, N], F32, kind="ExternalOutput")

    with tile.TileContext(nc) as tc:
        with (
            tc.tile_pool(name="const", bufs=1) as cpool,
            tc.tile_pool(name="big", bufs=2) as dpool,
            tc.tile_pool(name="small", bufs=3) as wpool,
            tc.tile_pool(name="ypsum", bufs=1, space="PSUM") as ypool,
        ):
            # ---- persistent SBUF tiles ----
            x_aug = cpool.tile([C + 1, N], F16, tag="xa")
            x_m = cpool.tile([C + 1, MH], F16, tag="xm")
            u_sb = cpool.tile([2 * CI, N], F16, tag="u")     # 2 row groups
            v_sb = cpool.tile([2 * CI, MH], F16, tag="v")
            gt_sb = cpool.tile([128, NBLK * CI], F32, tag="gt")
            wuT = cpool.tile([C + 1, CI], F16, tag="wu")
            wvT = cpool.tile([C + 1, CI], F16, tag="wv")
            wgT = cpool.tile([C + 1, CI], F16, tag="wg")
            wwT4 = cpool.tile([128, C], F16, tag="ww")
            resid = cpool.tile([C, N], F32, tag="resid")
            y_sbs = [cpool.tile([128, 4 * YCH], F16, tag=f"ysb{t}",
                                name=f"ysb{t}") for t in range(4)]

            # ---- input DMAs needed for the prologue (spread across
            # sequencers so nothing serializes at launch) ----
            nc.scalar.dma_start(wuT[:], wuT_d[:])
            nc.scalar.dma_start(wvT[:], wvT_d[:])
            nc.gpsimd.dma_start(wgT[:], wgT_d[:])
            for k in range(4):
                s = slice(k * (MH // 4), (k + 1) * (MH // 4))
                nc.gpsimd.dma_start(x_m[:, s], x_m_d[:, s])
            xa_edges = [0, 400, 800] + [800 * k for k in range(2, 9)]
            for k in range(len(xa_edges) - 1):
                s = slice(xa_edges[k], xa_edges[k + 1])
                nc.sync.dma_start(x_aug[:, s], x_aug_d[:, s])

            # ---- projections: u (full), v (this core's m range), g^T ----
            # projections borrow the y accumulator banks (their first
            # real matmul happens only at block 1): u rotates banks 0-1,
            # v rotates banks 2-3
            def emit_proj_u(k):
                off, cw = U_CHUNKS[k]
                pu = y_ps[k % 2][0:2 * CI, 0:512]
                for t in range(2):
                    nc.tensor.matmul(pu[CI * t:CI * (t + 1), :cw], wuT[:],
                                     x_aug[:, off:off + cw],
                                     start=True, stop=True,
                                     tile_position=(0, CI * t))
                if k % 2 == 0:
                    nc.scalar.copy(u_sb[:, off:off + cw], pu[:, :cw])
                else:
                    nc.vector.tensor_copy(u_sb[:, off:off + cw], pu[:, :cw])

            def emit_proj_v(k):
                off, cw = V_CHUNKS[k]
                pv = y_ps[2 + k % 2][0:2 * CI, 0:512]
                for t in range(2):
                    nc.tensor.matmul(pv[CI * t:CI * (t + 1), :cw], wvT[:],
                                     x_m[:, off:off + cw],
                                     start=True, stop=True,
                                     tile_position=(0, CI * t))
                nc.vector.tensor_copy(v_sb[:, off:off + cw], pv[:, :cw])

            # ---- y accumulators: 13 chunks packed 4-per-bank ----
            y_ps = [ypool.tile([128, YCH], F32, tag=f"y{t}", name=f"y{t}")
                    for t in range(4)]

            def y_slot(j):
                return y_ps[j // 4][32 * (j % 4):32 * (j % 4) + 32, :]

            with tc.tile_pool(name="spsum", bufs=2, space="PSUM") as spool:
                mm_state = [0]

                def emit_s_mms(i, ci):
                    """the 1-2 matmuls filling chunk ci of block i;
                    returns the psum tile"""
                    off, cw = CHUNKS[ci]
                    sp = spool.tile([128, SCH], F32, tag="s", name="sp")
                    for s2 in range(0, cw, 512):
                        w2 = min(512, cw - s2)
                        g = CI * (mm_state[0] % 2)  # alternate row groups
                        mm_state[0] += 1
                        nc.tensor.matmul(
                            sp[:, s2:s2 + w2],
                            v_sb[g:g + CI, i * MB:(i + 1) * MB],
                            u_sb[g:g + CI, off + s2:off + s2 + w2],
                            start=True, stop=True)
                    return sp

                def emit_exp_act(sp, ci, exp_t, sums):
                    off, cw = CHUNKS[ci]
                    nc.scalar.activation(
                        exp_t[:, off:off + cw].bitcast(BF16), sp[:, :cw], EXP,
                        accum_out=sums[:, ci:ci + 1])

                def emit_exp_dve(sp, ci, exp_t, sums):
                    off, cw = CHUNKS[ci]
                    # schraudolph: int16(S*A + B) -> bits of bf16 exp(S)
                    nc.vector.tensor_scalar(
                        exp_t[:, off:off + cw], sp[:, :cw], A_EXP, B_EXP,
                        mybir.AluOpType.mult, mybir.AluOpType.add)
                    # rowsum: in-place bf16 bypass with accum_out (4x mode)
                    ebf = exp_t[:, off:off + cw].bitcast(BF16)
                    nc.vector.tensor_scalar(
                        ebf, ebf, 1.0, None, mybir.AluOpType.mult,
                        mybir.AluOpType.add, accum_out=sums[:, ci:ci + 1])

                def emit_gts(i, sums):
                    tot = wpool.tile([128, 1], F32, tag="tot", name="tot")
                    nc.vector.tensor_reduce(tot[:], sums[:],
                                            mybir.AxisListType.X,
                                            mybir.AluOpType.add)
                    rec = wpool.tile([128, 1], F32, tag="rec", name="rec")
                    nc.vector.reciprocal(rec[:], tot[:])
                    gts = wpool.tile([128, CI], BF16, tag="gts", name="gts")
                    nc.vector.tensor_scalar_mul(
                        gts[:], gt_sb[:, i * CI:(i + 1) * CI], rec[:])
                    return gts

                def emit_y_mms(jlist, gts_prev, exp_prev, i_prev):
                    for j in jlist:
                        off, cw = Y_CHUNKS[j]
                        nc.tensor.matmul(
                            y_slot(j)[:, :cw], gts_prev,
                            exp_prev[:, off:off + cw].bitcast(BF16),
                            start=(i_prev == 0), stop=(i_prev == NBLK - 1),
                            tile_position=(0, 32 * (j % 4)),
                            skip_group_check=True)

                # ---- block 0: projections interleaved with its S chunks so
                # the first exp fires as early as possible
                exp0 = dpool.tile([128, N], I16, tag="expS", name="exp_t")
                sums0 = wpool.tile([128, NSUM], F32, tag="sums", name="sums")
                emit_proj_v(0)
                for ci in range(len(CHUNKS)):
                    for k in range(2 * ci, min(2 * ci + 2, len(U_CHUNKS))):
                        emit_proj_u(k)
                    sp = emit_s_mms(0, ci)
                    if ci in ACT_CHUNKS:
                        emit_exp_act(sp, ci, exp0, sums0)
                    else:
                        emit_exp_dve(sp, ci, exp0, sums0)
                exp_prev = exp0
                for k in range(1, len(V_CHUNKS)):
                    emit_proj_v(k)

                # g^T projections: PE fills while ACT crunches block 0.
                # psum outputs borrow the y accumulator banks (first y
                # matmul only happens at block 1); copies go to gpsimd.
                for i in range(NBLK):
                    pg = y_ps[i % 4][:, CI * (i // 4):CI * (i // 4 + 1)]
                    nc.tensor.matmul(pg, x_m[:, i * MB:(i + 1) * MB], wgT[:],
                                     start=True, stop=True,
                                     skip_group_check=True)
                # 4 strided copies (one per bank) instead of 25 small ones
                gt3 = gt_sb[:].rearrange("p (i c) -> p i c", c=CI)
                for b in range(4):
                    nslot = (NBLK - b + 3) // 4
                    nc.vector.tensor_copy(
                        gt3[:, b::4, :],
                        y_ps[b][:, 0:nslot * CI].rearrange(
                            "p (i c) -> p i c", c=CI))
                gts_prev = emit_gts(0, sums0)[:]

                # ---- main loop ----
                for i in range(1, NBLK):
                    exp_t = dpool.tile([128, N], I16, tag="expS", name="exp_t")
                    sums = wpool.tile([128, NSUM], F32, tag="sums",
                                      name="sums")
                    sps = {}
                    # S chunks 0-2, then y 0-5 of prev block, then S 3-6,
                    # then y 6-12: PE never starves while engines chew exps
                    for ci in (0, 1, 2):
                        sps[ci] = emit_s_mms(i, ci)
                    for ci in (0, 1, 2):
                        if ci in ACT_CHUNKS:
                            emit_exp_act(sps[ci], ci, exp_t, sums)
                        else:
                            emit_exp_dve(sps[ci], ci, exp_t, sums)
                    emit_y_mms(range(0, 6), gts_prev, exp_prev, i - 1)
                    for ci in (3, 4, 5, 6):
                        sps[ci] = emit_s_mms(i, ci)
                        if ci in ACT_CHUNKS:
                            emit_exp_act(sps[ci], ci, exp_t, sums)
                        else:
                            emit_exp_dve(sps[ci], ci, exp_t, sums)
                    emit_y_mms(range(6, 13), gts_prev, exp_prev, i - 1)

                    gts_prev = emit_gts(i, sums)[:]
                    exp_prev = exp_t

                # residual arrives while the main loop runs
                for k in range(4):
                    s = slice(k * (N // 4), (k + 1) * (N // 4))
                    nc.sync.dma_start(resid[:, s], resid_d[:, s])
                nc.sync.dma_start(wwT4[:], wwT4_d[:])

            # last block's y matmuls interleaved with the drain pipeline
            # per y bank: the drains for bank b (psum->f16 copy on ACT,
            # Ww matmul, resid add on DVE, store) start after only that
            # bank's 4 y matmuls instead of all 13
            with tc.tile_pool(name="fpsum", bufs=4, space="PSUM") as fpool:
                i = NBLK - 1

                def emit_last_y(jlist):
                    for j in jlist:
                        off, cw = Y_CHUNKS[j]
                        nc.tensor.matmul(
                            y_slot(j)[:, :cw], gts_prev,
                            exp_prev[:, off:off + cw].bitcast(BF16),
                            start=(i == 0), stop=True,
                            tile_position=(0, 32 * (j % 4)),
                            skip_group_check=True)

                def emit_drains(jlist):
                    for j in jlist:
                        off, cw = Y_CHUNKS[j]
                        p = 32 * (j % 4)
                        ys = y_sbs[j % 4][p:p + 32,
                                          (j // 4) * YCH:(j // 4) * YCH + cw]
                        nc.scalar.copy(ys, y_slot(j)[:, :cw])
                        fp = fpool.tile([C, YCH], F32, tag="f")
                        nc.tensor.matmul(fp[:, :cw], wwT4[p:p + 32, :], ys,
                                         start=True, stop=True,
                                         tile_position=(p, 0))
                        ot = wpool.tile([C, YCH], F32, tag="ot")
                        nc.vector.tensor_add(
                            ot[:, :cw], fp[:, :cw], resid[:, off:off + cw])
                        nc.sync.dma_start(out_d[:, off:off + cw],
                                          ot[:, :cw])

                emit_last_y(range(0, 4))
                emit_last_y(range(4, 8))
                emit_drains(range(0, 4))
                emit_last_y(range(8, 12))
                emit_drains(range(4, 8))
                emit_last_y(range(12, 13))
                emit_drains(range(8, 13))

    nc.compile()
    return nc


def make_in_maps(x, Wg, bg, Wu, bu, Wv, bv, Ww, bw):
    x = np.asarray(x, np.float32)
    x16 = x.astype(np.float16)
    ones = np.ones((1, N), np.float16)
    wuT = np.concatenate([np.asarray(Wu, np.float32).T,
                          np.asarray(bu, np.float32)[None, :]], 0).astype(np.float16)
    wvT = np.concatenate([np.asarray(Wv, np.float32).T,
                          np.asarray(bv, np.float32)[None, :]], 0).astype(np.float16)
    wgT = np.concatenate([np.asarray(Wg, np.float32).T,
                          np.asarray(bg, np.float32)[None, :]], 0).astype(np.float16)
    wwT4 = np.concatenate(
        [np.ascontiguousarray(np.asarray(Ww, np.float32).T)] * 4, 0).astype(np.float16)
    bw = np.asarray(bw, np.float32)

    in_maps = []
    for core in range(NCORES):
        b, h = divmod(core, 2)
        xb16 = x16[b].reshape(C, N)
        x_aug = np.concatenate([xb16, ones], 0)
        x_m = np.ascontiguousarray(x_aug[:, h * MH:(h + 1) * MH])
        if h == 1:
            residc = x[b].reshape(C, N) + bw[:, None]
        else:
            residc = np.zeros((C, N), np.float32)
        in_maps.append({
            "x_aug": np.ascontiguousarray(x_aug),
            "x_m": x_m,
            "wuT": np.ascontiguousarray(wuT),
            "wvT": np.ascontiguousarray(wvT),
            "wgT": np.ascontiguousarray(wgT),
            "wwT4": np.ascontiguousarray(wwT4),
            "resid": np.ascontiguousarray(residc),
        })
    return in_maps


_NC = None


def kernel(x, Wg, bg, Wu, bu, Wv, bv, Ww, bw, _trace=False):
    global _NC
    if _NC is None:
        _NC = build_nc()
    in_maps = make_in_maps(x, Wg, bg, Wu, bu, Wv, bv, Ww, bw)
    res = run_bass_kernel_spmd(_NC, in_maps, list(range(NCORES)), trace=_trace)
    outs = [r["out"] for r in res.results]
    full = np.empty((B, C, H, W), np.float32)
    for b in range(B):
        full[b] = (outs[2 * b] + outs[2 * b + 1]).reshape(C, H, W)
    kernel.last_results = res
    return full


if __name__ == "__main__":
    rng = np.random.default_rng(0)
    s_in, s_mid = 1.0 / np.sqrt(C), 1.0 / np.sqrt(CI)
    ins = dict(
        x=rng.standard_normal((B, C, H, W), np.float32),
        Wg=(rng.standard_normal((CI, C)) * s_in).astype(np.float32),
        bg=(rng.standard_normal(CI) * 0.01).astype(np.float32),
        Wu=(rng.standard_normal((CI, C)) * s_in).astype(np.float32),
        bu=(rng.standard_normal(CI) * 0.01).astype(np.float32),
        Wv=(rng.standard_normal((CI, C)) * s_in).astype(np.float32),
        bv=(rng.standard_normal(CI) * 0.01).astype(np.float32),
        Ww=(rng.standard_normal((C, CI)) * s_mid).astype(np.float32),
        bw=(rng.standard_normal(C) * 0.01).astype(np.float32),
    )
    out = kernel(**ins)
    print("kernel output", out.shape, out.dtype)


# revision 24
# speedup vs baseline: 1.0446x; 1.0446x over previous
# Trainium2 Bass kernel for NonLocalBlock (B=4, C=64, CI=32, H=W=80).
#
# Math (per batch, N = H*W = 6400):
#   u = Wu@x+bu, v = Wv@x+bv, g = Wg@x+bg           [CI, N]
#   f[n,m] = sum_c u[c,n] v[c,m]; softmax over n (axis=1 of f)
#   y[c,n] = sum_m f_sm[n,m] g[c,m];  out = Ww@y + bw + x
#
# S = v^T u (S[m,n] = f[n,m]); softmax is row-local over S's free axis.
# y = g @ softmax_rows(S).  8 cores = 4 batches x 2 halves of m.
#
# v2: the exp work (20.5M elems/core) is split between ACT and DVE.
#  - ACT-owned chunks: exact exp via activation(Exp), rowsums from accum_out.
#  - DVE-owned chunks: Schraudolph bit-trick exp at 1 elem/cycle:
#      int16(S * 128/ln2 + MAGIC)  bitcast->  bf16 ~= exp(S)
#    (bf16's exponent field absorbs the integer part; the mantissa linearly
#    interpolates 2^frac; MAGIC centers the +-3.3% sawtooth error).
#    Rowsums for these chunks come from a second bf16->bf16 bypass
#    tensor_scalar with accum_out, which runs in DVE 4x mode (0.25 cyc/elem).
#  - Softmax normalization divides by the sum of the SAME approx values, so
#    the sawtooth error partially cancels; end-to-end rel err ~3e-3.
# Per-block engine budget ~4.9-5.2us vs PE ~5.3us (S+y matmuls at 1 col/cyc
# fp16/bf16): roughly balanced, PE-bound at full clock.

import numpy as np

import concourse.bass as bass
import concourse.mybir as mybir
from concourse import bacc, tile
from concourse.bass_utils import run_bass_kernel_spmd

F32 = mybir.dt.float32
BF16 = mybir.dt.bfloat16
F16 = mybir.dt.float16
I16 = mybir.dt.int16

B, C, CI, H, W = 4, 64, 32, 80, 80
N = H * W              # 6400
NCORES = 8
MH = N // 2            # 3200 rows of S per core
MB = 128               # S row-block
NBLK = MH // MB        # 25 blocks per core
SCH = 1024             # S free-dim chunk held in PSUM (2 banks)
YCH = 512              # y free-dim chunk (1 bank)

EXP = mybir.ActivationFunctionType.Exp
A_EXP = float(np.float32(128.0 / np.log(2.0)))
B_EXP = float(np.float32(16250.875))

# chunk layout: 6x1024 + 256 tail.  Ownership tuned to measured HW costs:
# ACT-owned 1024-chunk = exp ~1121ns + accum-read ~285ns; DVE-owned =
# schraudolph ts ~1222ns (PSUM f32 in, 1x) + bypass-accum ~1210ns (the
# accumulating tensor_scalar never engages 2x/4x DVE perf modes on HW),
# so ACT rowsums are ~4x cheaper and ACT owns more columns.  ACT takes
# {0,1,4,5,tail} and DVE {2,3} so each psum ping-pong tile carries a
# balanced mix of ACT and DVE exps (the per-tile fill->exp->refill chain
# is the critical path).
CHUNKS = [(k * SCH, SCH) for k in range(6)] + [(6 * SCH, 256)]
ACT_CHUNKS = [0, 1, 4, 5, 6]
DVE_CHUNKS = [2, 3]
NSUM = len(CHUNKS)


def _ceil_chunks(total, step):
    out = []
    off = 0
    while off < total:
        out.append((off, min(step, total - off)))
        off += step
    return out


Y_CHUNKS = _ceil_chunks(N, YCH)      # 12 x 512 + 256
U_CHUNKS = _ceil_chunks(N, 512)
V_CHUNKS = _ceil_chunks(MH, 512)


def build_nc():
    nc = bacc.Bacc("TRN2", target_bir_lowering=False, debug=False,
                   num_devices=NCORES)

    x_aug_d = nc.dram_tensor("x_aug", [C + 1, N], F16, kind="ExternalInput")
    x_m_d = nc.dram_tensor("x_m", [C + 1, MH], F16, kind="ExternalInput")
    wuT_d = nc.dram_tensor("wuT", [C + 1, CI], F16, kind="ExternalInput")
    wvT_d = nc.dram_tensor("wvT", [C + 1, CI], F16, kind="ExternalInput")
    wgT_d = nc.dram_tensor("wgT", [C + 1, CI], F16, kind="ExternalInput")
    wwT4_d = nc.dram_tensor("wwT4", [128, C], F16, kind="ExternalInput")
    resid_d = nc.dram_tensor("resid", [C, N], F32, kind="ExternalInput")
    out_d = nc.dram_tensor("out", [C, N], F32, kind="ExternalOutput")

    with tile.TileContext(nc) as tc:
        with (
            tc.tile_pool(name="const", bufs=1) as cpool,
            tc.tile_pool(name="big", bufs=2) as dpool,
            tc.tile_pool(name="small", bufs=3) as wpool,
            tc.tile_pool(name="ypsum", bufs=1, space="PSUM") as ypool,
        ):
            # ---- persistent SBUF tiles ----
            x_aug = cpool.tile([C + 1, N], F16, tag="xa")
            x_m = cpool.tile([C + 1, MH], F16, tag="xm")
            u_sb = cpool.tile([2 * CI, N], F16, tag="u")     # 2 row groups
            v_sb = cpool.tile([2 * CI, MH], F16, tag="v")
            gt_sb = cpool.tile([128, NBLK * CI], F32, tag="gt")
            wuT = cpool.tile([C + 1, CI], F16, tag="wu")
            wvT = cpool.tile([C + 1, CI], F16, tag="wv")
            wgT = cpool.tile([C + 1, CI], F16, tag="wg")
            wwT4 = cpool.tile([128, C], F16, tag="ww")
            resid = cpool.tile([C, N], F32, tag="resid")
            y_sbs = [cpool.tile([128, 4 * YCH], F16, tag=f"ysb{t}",
                                name=f"ysb{t}") for t in range(4)]

            # ---- input DMAs needed for the prologue (spread across
            # sequencers so nothing serializes at launch) ----
            nc.scalar.dma_start(wuT[:], wuT_d[:])
            nc.scalar.dma_start(wvT[:], wvT_d[:])
            nc.gpsimd.dma_start(wgT[:], wgT_d[:])
            for k in range(4):
                s = slice(k * (MH // 4), (k + 1) * (MH // 4))
                nc.gpsimd.dma_start(x_m[:, s], x_m_d[:, s])
            xa_edges = [0, 400, 800] + [800 * k for k in range(2, 9)]
            for k in range(len(xa_edges) - 1):
                s = slice(xa_edges[k], xa_edges[k + 1])
                nc.sync.dma_start(x_aug[:, s], x_aug_d[:, s])

            # ---- projections: u (full), v (this core's m range), g^T ----
            # projections borrow the y accumulator banks (their first
            # real matmul happens only at block 1): u rotates banks 0-1,
            # v rotates banks 2-3
            def emit_proj_u(k):
                off, cw = U_CHUNKS[k]
                pu = y_ps[k % 2][0:2 * CI, 0:512]
                for t in range(2):
                    nc.tensor.matmul(pu[CI * t:CI * (t + 1), :cw], wuT[:],
                                     x_aug[:, off:off + cw],
                                     start=True, stop=True,
                                     tile_position=(0, CI * t))
                if k % 2 == 0:
                    nc.scalar.copy(u_sb[:, off:off + cw], pu[:, :cw])
                else:
                    nc.vector.tensor_copy(u_sb[:, off:off + cw], pu[:, :cw])

            def emit_proj_v(k):
                off, cw = V_CHUNKS[k]
                pv = y_ps[2 + k % 2][0:2 * CI, 0:512]
                for t in range(2):
                    nc.tensor.matmul(pv[CI * t:CI * (t + 1), :cw], wvT[:],
                                     x_m[:, off:off + cw],
                                     start=True, stop=True,
                                     tile_position=(0, CI * t))
                nc.vector.tensor_copy(v_sb[:, off:off + cw], pv[:, :cw])

            # ---- y accumulators: 13 chunks packed 4-per-bank ----
            y_ps = [ypool.tile([128, YCH], F32, tag=f"y{t}", name=f"y{t}")
                    for t in range(4)]

            def y_slot(j):
                return y_ps[j // 4][32 * (j % 4):32 * (j % 4) + 32, :]

            with tc.tile_pool(name="spsum", bufs=2, space="PSUM") as spool:
                mm_state = [0]

                def emit_s_mms(i, ci):
                    """the 1-2 matmuls filling chunk ci of block i;
                    returns the psum tile"""
                    off, cw = CHUNKS[ci]
                    sp = spool.tile([128, SCH], F32, tag="s", name="sp")
                    for s2 in range(0, cw, 512):
                        w2 = min(512, cw - s2)
                        g = CI * (mm_state[0] % 2)  # alternate row groups
                        mm_state[0] += 1
                        nc.tensor.matmul(
                            sp[:, s2:s2 + w2],
                            v_sb[g:g + CI, i * MB:(i + 1) * MB],
                            u_sb[g:g + CI, off + s2:off + s2 + w2],
                            start=True, stop=True)
                    return sp

                def emit_exp_act(sp, ci, exp_t, sums):
                    off, cw = CHUNKS[ci]
                    nc.scalar.activation(
                        exp_t[:, off:off + cw].bitcast(BF16), sp[:, :cw], EXP,
                        accum_out=sums[:, ci:ci + 1])

                def emit_exp_dve(sp, ci, exp_t, sums):
                    off, cw = CHUNKS[ci]
                    # schraudolph: int16(S*A + B) -> bits of bf16 exp(S)
                    nc.vector.tensor_scalar(
                        exp_t[:, off:off + cw], sp[:, :cw], A_EXP, B_EXP,
                        mybir.AluOpType.mult, mybir.AluOpType.add)
                    # rowsum: in-place bf16 bypass with accum_out (4x mode)
                    ebf = exp_t[:, off:off + cw].bitcast(BF16)
                    nc.vector.tensor_scalar(
                        ebf, ebf, 1.0, None, mybir.AluOpType.mult,
                        mybir.AluOpType.add, accum_out=sums[:, ci:ci + 1])

                def emit_gts(i, sums):
                    tot = wpool.tile([128, 1], F32, tag="tot", name="tot")
                    nc.vector.tensor_reduce(tot[:], sums[:],
                                            mybir.AxisListType.X,
                                            mybir.AluOpType.add)
                    rec = wpool.tile([128, 1], F32, tag="rec", name="rec")
                    nc.vector.reciprocal(rec[:], tot[:])
                    gts = wpool.tile([128, CI], BF16, tag="gts", name="gts")
                    nc.vector.tensor_scalar_mul(
                        gts[:], gt_sb[:, i * CI:(i + 1) * CI], rec[:])
                    return gts

                def emit_y_mms(jlist, gts_prev, exp_prev, i_prev):
                    for j in jlist:
                        off, cw = Y_CHUNKS[j]
                        nc.tensor.matmul(
                            y_slot(j)[:, :cw], gts_prev,
                            exp_prev[:, off:off + cw].bitcast(BF16),
                            start=(i_prev == 0), stop=(i_prev == NBLK - 1),
                            tile_position=(0, 32 * (j % 4)),
                            skip_group_check=True)

                # ---- block 0: projections interleaved with its S chunks so
                # the first exp fires as early as possible
                exp0 = dpool.tile([128, N], I16, tag="expS", name="exp_t")
                sums0 = wpool.tile([128, NSUM], F32, tag="sums", name="sums")
                emit_proj_v(0)
                for ci in range(len(CHUNKS)):
                    for k in range(2 * ci, min(2 * ci + 2, len(U_CHUNKS))):
                        emit_proj_u(k)
                    sp = emit_s_mms(0, ci)
                    if ci in ACT_CHUNKS:
                        emit_exp_act(sp, ci, exp0, sums0)
                    else:
                        emit_exp_dve(sp, ci, exp0, sums0)
                exp_prev = exp0
                for k in range(1, len(V_CHUNKS)):
                    emit_proj_v(k)

                # g^T projections: PE fills while ACT crunches block 0.
                # psum outputs borrow the y accumulator banks (first y
                # matmul only happens at block 1); copies go to gpsimd.
                for i in range(NBLK):
                    pg = y_ps[i % 4][:, CI * (i // 4):CI * (i // 4 + 1)]
                    nc.tensor.matmul(pg, x_m[:, i * MB:(i + 1) * MB], wgT[:],
                                     start=True, stop=True,
                                     skip_group_check=True)
                # 4 strided copies (one per bank) instead of 25 small ones
                gt3 = gt_sb[:].rearrange("p (i c) -> p i c", c=CI)
                for b in range(4):
                    nslot = (NBLK - b + 3) // 4
                    nc.vector.tensor_copy(
                        gt3[:, b::4, :],
                        y_ps[b][:, 0:nslot * CI].rearrange(
                            "p (i c) -> p i c", c=CI))
                gts_prev = emit_gts(0, sums0)[:]

                # ---- main loop ----
                for i in range(1, NBLK):
                    exp_t = dpool.tile([128, N], I16, tag="expS", name="exp_t")
                    sums = wpool.tile([128, NSUM], F32, tag="sums",
                                      name="sums")
                    sps = {}
                    # S chunks 0-2, then y 0-5 of prev block, then S 3-6,
                    # then y 6-12: PE never starves while engines chew exps
                    for ci in (0, 1, 2):
                        sps[ci] = emit_s_mms(i, ci)
                    for ci in (0, 1, 2):
                        if ci in ACT_CHUNKS:
                            emit_exp_act(sps[ci], ci, exp_t, sums)
                        else:
                            emit_exp_dve(sps[ci], ci, exp_t, sums)
                    emit_y_mms(range(0, 6), gts_prev, exp_prev, i - 1)
                    for ci in (3, 4, 5, 6):
                        sps[ci] = emit_s_mms(i, ci)
                        if ci in ACT_CHUNKS:
                            emit_exp_act(sps[ci], ci, exp_t, sums)
                        else:
                            emit_exp_dve(sps[ci], ci, exp_t, sums)
                    emit_y_mms(range(6, 13), gts_prev, exp_prev, i - 1)

                    gts_prev = emit_gts(i, sums)[:]
                    exp_prev = exp_t

                # residual arrives while the main loop runs
                for k in range(4):
                    s = slice(k * (N // 4), (k + 1) * (N // 4))
                    nc.sync.dma_start(resid[:, s], resid_d[:, s])
                nc.sync.dma_start(wwT4[:], wwT4_d[:])

            # last block's y matmuls interleaved with the drain pipeline
            # per y bank: the drains for bank b (psum->f16 copy on ACT,
            # Ww matmul, resid add on DVE, store) start after only that
            # bank's 4 y matmuls instead of all 13
            with tc.tile_pool(name="fpsum", bufs=4, space="PSUM") as fpool:
                i = NBLK - 1

                def emit_last_y(jlist):
                    for j in jlist:
                        off, cw = Y_CHUNKS[j]
                        nc.tensor.matmul(
                            y_slot(j)[:, :cw], gts_prev,
                            exp_prev[:, off:off + cw].bitcast(BF16),
                            start=(i == 0), stop=True,
                            tile_position=(0, 32 * (j % 4)),
                            skip_group_check=True)

                def emit_drains(jlist):
                    for j in jlist:
                        off, cw = Y_CHUNKS[j]
                        p = 32 * (j % 4)
                        ys = y_sbs[j % 4][p:p + 32,
                                          (j // 4) * YCH:(j // 4) * YCH + cw]
                        nc.scalar.copy(ys, y_slot(j)[:, :cw])
                        fp = fpool.tile([C, YCH], F32, tag="f")
                        nc.tensor.matmul(fp[:, :cw], wwT4[p:p + 32, :], ys,
                                         start=True, stop=True,
                                         tile_position=(p, 0))
                        ot = wpool.tile([C, YCH], F32, tag="ot")
                        nc.vector.tensor_add(
                            ot[:, :cw], fp[:, :cw], resid[:, off:off + cw])
                        nc.sync.dma_start(out_d[:, off:off + cw],
                                          ot[:, :cw])

                emit_last_y(range(0, 4))
                emit_last_y(range(4, 8))
                emit_drains(range(0, 4))
                emit_last_y(range(8, 12))
                emit_drains(range(4, 8))
                emit_last_y(range(12, 13))
                emit_drains(range(8, 13))

    nc.compile()
    return nc


def make_in_maps(x, Wg, bg, Wu, bu, Wv, bv, Ww, bw):
    x = np.asarray(x, np.float32)
    x16 = x.astype(np.float16)
    ones = np.ones((1, N), np.float16)
    wuT = np.concatenate([np.asarray(Wu, np.float32).T,
                          np.asarray(bu, np.float32)[None, :]], 0).astype(np.float16)
    wvT = np.concatenate([np.asarray(Wv, np.float32).T,
                          np.asarray(bv, np.float32)[None, :]], 0).astype(np.float16)
    wgT = np.concatenate([np.asarray(Wg, np.float32).T,
                          np.asarray(bg, np.float32)[None, :]], 0).astype(np.float16)
    wwT4 = np.concatenate(
        [np.ascontiguousarray(np.asarray(Ww, np.float32).T)] * 4, 0).astype(np.float16)
    bw = np.asarray(bw, np.float32)

    in_maps = []
    for core in range(NCORES):
        b, h = divmod(core, 2)
        xb16 = x16[b].reshape(C, N)
        x_aug = np.concatenate([xb16, ones], 0)
        x_m = np.ascontiguousarray(x_aug[:, h * MH:(h + 1) * MH])
        if h == 1:
            residc = x[b].reshape(C, N) + bw[:, None]
        else:
            residc = np.zeros((C, N), np.float32)
        in_maps.append({
            "x_aug": np.ascontiguousarray(x_aug),
            "x_m": x_m,
            "wuT": np.ascontiguousarray(wuT),
            "wvT": np.ascontiguousarray(wvT),
            "wgT": np.ascontiguousarray(wgT),
            "wwT4": np.ascontiguousarray(wwT4),
            "resid": np.ascontiguousarray(residc),
        })
    return in_maps


_NC = None


def kernel(x, Wg, bg, Wu, bu, Wv, bv, Ww, bw, _trace=False):
    global _NC
    if _NC is None:
        _NC = build_nc()
    in_maps = make_in_maps(x, Wg, bg, Wu, bu, Wv, bv, Ww, bw)
    res = run_bass_kernel_spmd(_NC, in_maps, list(range(NCORES)), trace=_trace)
    outs = [r["out"] for r in res.results]
    full = np.empty((B, C, H, W), np.float32)
    for b in range(B):
        full[b] = (outs[2 * b] + outs[2 * b + 1]).reshape(C, H, W)
    kernel.last_results = res
    return full


if __name__ == "__main__":
    rng = np.random.default_rng(0)
    s_in, s_mid = 1.0 / np.sqrt(C), 1.0 / np.sqrt(CI)
    ins = dict(
        x=rng.standard_normal((B, C, H, W), np.float32),
        Wg=(rng.standard_normal((CI, C)) * s_in).astype(np.float32),
        bg=(rng.standard_normal(CI) * 0.01).astype(np.float32),
        Wu=(rng.standard_normal((CI, C)) * s_in).astype(np.float32),
        bu=(rng.standard_normal(CI) * 0.01).astype(np.float32),
        Wv=(rng.standard_normal((CI, C)) * s_in).astype(np.float32),
        bv=(rng.standard_normal(CI) * 0.01).astype(np.float32),
        Ww=(rng.standard_normal((C, CI)) * s_mid).astype(np.float32),
        bw=(rng.standard_normal(C) * 0.01).astype(np.float32),
    )
    out = kernel(**ins)
    print("kernel output", out.shape, out.dtype)
